# revision 1
# baseline (speedup 1.0000x reference)
"""Trainium2 Bass kernel: 2-layer GraphSAGE (mean aggregation), 8-core SPMD.

nn_BiGNN: out = sage2(relu(sage1(x)));  sage(x) = mean_{j->i}(x_j) @ W_l + b_l + x @ W_r
N=50000 nodes, E=800000 edges, d=128, f32 inputs / f32 output.

Strategy (one NeuronCore owns 6250 destination nodes):
  - host: partition edges by destination block, sort by dst, pad per
    128-dst subwindow, split into src<32768 / src>=32768 phases (int16
    SWDGE index limit), equalize batch counts across cores (SPMD).
  - device: SWDGE dma_gather of bf16 source rows; one-hot segment matrices
    on DVE; TensorE matmul msg^T @ seg accumulated per 512-node PSUM
    window = transposed mean-aggregation; 1/deg scaling folded into the
    PSUM evacuation; weight matmuls + bias + relu; AllGather of h between
    layers; final layer emits row-major output directly.
"""

import os
import sys
import types

for _p in ("/opt/trn_rl_repo", "/root/.axon_site/_ro/trn_rl_repo",
           "/root/.axon_site"):
    if os.path.isdir(_p) and _p not in sys.path:
        sys.path.insert(0, _p)


def _install_ntff_hook():
    """Provide antenv.axon_hooks (missing in this image) so trace=True can
    capture NTFF profiles through libaxon_pjrt.so."""
    if "antenv.axon_hooks" in sys.modules:
        return
    store = [None]
    mod = types.ModuleType("antenv.axon_hooks")
    mod.set_axon_ntff_profile_hook = lambda h: store.__setitem__(0, h)
    mod.get_axon_ntff_profile_hook = lambda: store[0]
    sys.modules["antenv.axon_hooks"] = mod
    try:
        import antenv
        antenv.axon_hooks = mod
        from trn_agent_boot.trn_boot import _ntff_profile_via_ctypes
        so = "/opt/axon/libaxon_pjrt.so"
        if os.path.exists(so):
            mod.set_axon_ntff_profile_hook(_ntff_profile_via_ctypes(so))
    except Exception:
        pass


_install_ntff_hook()


import numpy as np
import ml_dtypes

import concourse.bass as bass
import concourse.bacc as bacc
import concourse.mybir as mybir
import concourse.tile as tile
from concourse.library_config import mlp as mlp_library

P = 128
D = 128
GMAX = 8  # max batches (1024 idxs) per dma_gather instruction
HALF = 32768  # int16 index limit for dma_gather
F32 = mybir.dt.float32
BF16 = mybir.dt.bfloat16
I16 = mybir.dt.int16


def wrap_idx16(arr):
    """[n] int array -> [128, n//16] int16 SWDGE layout (16-partition wrap,
    replicated for the 8 Q7 cores)."""
    n = arr.shape[0]
    assert n % 16 == 0
    w = np.asarray(arr, dtype=np.int16).reshape(n // 16, 16).T  # [16, n/16]
    return np.tile(w, (8, 1))  # [128, n/16]


# ----------------------------------------------------------------- host prep
def host_prep(edge_index, n_nodes, ncores, win=512):
    """Partition/sort/pad edges; build the phased stream + SWDGE idx arrays.

    Stream order: for each window w: [lo-phase batches][hi-phase batches],
    each phase grouped by subwindow. Per-(sub, phase) batch counts are
    equalized across cores so all cores share one program.
    """
    npc = n_nodes // ncores
    nsub = (npc + P - 1) // P
    nwin = (npc + win - 1) // win
    spw = win // P  # subwindows per window
    src_a = np.asarray(edge_index[0], dtype=np.int64)
    dst_a = np.asarray(edge_index[1], dtype=np.int64)

    # per (core, sub, phase) edge lists
    edges = [[None] * (2 * nsub) for _ in range(ncores)]
    invcnt = np.zeros((ncores, 1, npc), dtype=np.float32)
    for c in range(ncores):
        lo_n = c * npc
        m = (dst_a >= lo_n) & (dst_a < lo_n + npc)
        s, d = src_a[m], dst_a[m] - lo_n
        invcnt[c, 0] = 1.0 / np.maximum(np.bincount(d, minlength=npc), 1.0)
        order = np.argsort(d, kind="stable")
        s, d = s[order], d[order]
        sub = d // P
        for t in range(nsub):
            ms = sub == t
            st, dt_ = s[ms], d[ms]
            mlo = st < HALF
            edges[c][2 * t] = (st[mlo], dt_[mlo])               # lo phase
            edges[c][2 * t + 1] = (st[~mlo] - HALF, dt_[~mlo])  # hi phase

    # equalized batch counts per (sub, phase)
    nb = np.zeros((nsub, 2), dtype=np.int64)
    for c in range(ncores):
        for t in range(nsub):
            for ph in range(2):
                n = len(edges[c][2 * t + ph][0])
                nb[t, ph] = max(nb[t, ph], (n + P - 1) // P)
    nb[:, 0] = np.maximum(nb[:, 0], 1)  # ensure each sub has >=1 batch

    # stream layout: per window w, phase ph: one gather block
    blocks = []
    ncols = 0
    for w in range(nwin):
        subs = range(w * spw, min((w + 1) * spw, nsub))
        for ph in range(2):
            bl = [(t, int(nb[t, ph])) for t in subs]
            nbl = sum(x[1] for x in bl)
            blocks.append(dict(w=w, ph=ph, col0=ncols, nb=nbl, subs=bl))
            ncols += nbl

    # per-core data arrays in stream order
    idx16 = np.zeros((ncores, P, ncols * 8), dtype=np.int16)  # nb*128/16 = nb*8
    slots = np.full((ncores, P, ncols), -1.0, dtype=ml_dtypes.bfloat16)
    for c in range(ncores):
        for blk in blocks:
            if blk["nb"] == 0:
                continue
            col = blk["col0"]
            flat_idx = []
            for t, nbt in blk["subs"]:
                s, d = edges[c][2 * t + blk["ph"]]
                n = len(s)
                npad = nbt * P
                si = np.zeros(npad, dtype=np.int64)
                si[:n] = s
                sl = np.full(npad, -1.0, dtype=np.float32)
                sl[:n] = (d % P).astype(np.float32)
                for b in range(nbt):
                    slots[c, :, col + b] = sl[b * P:(b + 1) * P].astype(
                        ml_dtypes.bfloat16)
                flat_idx.append(si)
                col += nbt
            fi = np.concatenate(flat_idx)
            idx16[c, :, blk["col0"] * 8:(blk["col0"] + blk["nb"]) * 8] = \
                wrap_idx16(fi)

    return dict(npc=npc, nsub=nsub, nwin=nwin, win=win, ncols=ncols,
                blocks=blocks, idx16=idx16, slots=slots, invcnt=invcnt)


# -------------------------------------------------------------- kernel build
def build_kernel(n_nodes, ncores, prep, nb_onehot=8, fuse_onehot=False):
    npc, nwin, win = prep["npc"], prep["nwin"], prep["win"]
    ncols, blocks = prep["ncols"], prep["blocks"]
    spw = win // P

    nc = bacc.Bacc(None)

    xtab = nc.declare_dram_parameter("xtab", [n_nodes, D], BF16, isOutput=False)
    xT_d = nc.declare_dram_parameter("xT", [D, npc], F32, isOutput=False)
    idx_d = nc.declare_dram_parameter("idx16", [P, ncols * 8], I16, isOutput=False)
    slots_d = nc.declare_dram_parameter("slots", [P, ncols], BF16, isOutput=False)
    invcnt_d = nc.declare_dram_parameter("invcnt", [P, npc], F32, isOutput=False)
    W1l_d = nc.declare_dram_parameter("W1l", [D, D], F32, isOutput=False)
    W1r_d = nc.declare_dram_parameter("W1r", [D, D], F32, isOutput=False)
    W2l_d = nc.declare_dram_parameter("W2l", [D, D], F32, isOutput=False)
    W2r_d = nc.declare_dram_parameter("W2r", [D, D], F32, isOutput=False)
    b1_d = nc.declare_dram_parameter("b1", [D, 1], F32, isOutput=False)
    b2row_d = nc.declare_dram_parameter("b2row", [P, D], F32, isOutput=False)
    iota_d = nc.declare_dram_parameter("iota", [P, P], BF16, isOutput=False)
    ident_d = nc.declare_dram_parameter("ident", [P, P], F32, isOutput=False)
    out_d = nc.declare_dram_parameter("out", [npc, D], F32, isOutput=True)


    from contextlib import ExitStack
    with tile.TileContext(nc) as tc, ExitStack() as es:
        dram = es.enter_context(tc.tile_pool(name="dram", bufs=1, space="DRAM"))
        h_local = dram.tile([npc, D], BF16, tag="hloc")
        h_full = dram.tile([n_nodes, D], BF16, tag="hfull", addr_space="Shared")

        const = es.enter_context(tc.tile_pool(name="const", bufs=1))
        sb = es.enter_context(tc.tile_pool(name="sb", bufs=1))
        msgp = es.enter_context(tc.tile_pool(name="msgp", bufs=6))
        segp = es.enter_context(tc.tile_pool(name="segp", bufs=4))
        aggp = es.enter_context(tc.tile_pool(name="aggp", bufs=2))
        rowp = es.enter_context(tc.tile_pool(name="rowp", bufs=3))
        psA = es.enter_context(tc.tile_pool(name="psA", bufs=2, space="PSUM"))
        psB = es.enter_context(tc.tile_pool(name="psB", bufs=2, space="PSUM"))
        psT = es.enter_context(tc.tile_pool(name="psT", bufs=2, space="PSUM"))

        nc.gpsimd.load_library(mlp_library)

        idx_sb = const.tile([P, ncols * 8], I16, tag="idx")
        slots_sb = const.tile([P, ncols], BF16, tag="slots")
        invcnt_sb = const.tile([P, npc], F32, tag="invcnt")
        iota_sb = const.tile([P, P], BF16, tag="iota")
        ident_sb = const.tile([P, P], F32, tag="ident")
        W1l_sb = const.tile([D, D], F32, tag="W1l")
        W1r_sb = const.tile([D, D], F32, tag="W1r")
        W2l_sb = const.tile([D, D], F32, tag="W2l")
        W2r_sb = const.tile([D, D], F32, tag="W2r")
        b1_sb = const.tile([D, 1], F32, tag="b1")
        b2row_sb = const.tile([P, D], F32, tag="b2row")
        xT_sb = sb.tile([D, npc], F32, tag="xT")
        hT_sb = sb.tile([D, npc], F32, tag="hT")

        for t, dd in [(idx_sb, idx_d), (slots_sb, slots_d), (invcnt_sb, invcnt_d),
                      (iota_sb, iota_d), (ident_sb, ident_d),
                      (W1l_sb, W1l_d), (W1r_sb, W1r_d), (W2l_sb, W2l_d),
                      (W2r_sb, W2r_d), (b1_sb, b1_d), (b2row_sb, b2row_d),
                      (xT_sb, xT_d)]:
            nc.sync.dma_start(out=t[:], in_=dd[:])

        ngrp = (ncols + nb_onehot - 1) // nb_onehot

        def emit_layer(layer, table):
            segs = []
            for g in range(ngrp):
                nbg = min(nb_onehot, ncols - g * nb_onehot)
                seg = segp.tile([P, nb_onehot, P], BF16, tag="seg",
                                name=f"seg{layer}_{g}")
                g0 = g * nb_onehot
                if fuse_onehot:
                    nc.vector.tensor_tensor(
                        out=seg[:, :nbg, :],
                        in0=iota_sb[:, None, :].to_broadcast([P, nbg, P]),
                        in1=slots_sb[:, g0:g0 + nbg, None].to_broadcast([P, nbg, P]),
                        op=mybir.AluOpType.is_equal,
                    )
                else:
                    for bi in range(nbg):
                        nc.vector.tensor_tensor(
                            out=seg[:, bi, :],
                            in0=iota_sb[:],
                            in1=slots_sb[:, g0 + bi:g0 + bi + 1].to_broadcast([P, P]),
                            op=mybir.AluOpType.is_equal,
                        )
                segs.append(seg)

            tab_lo = table[0:min(HALF, n_nodes), :]
            tab_hi = table[HALF:n_nodes, :] if n_nodes > HALF else None

            for w in range(nwin):
                n0 = w * win
                wn = min(win, npc - n0)
                nsw = (wn + P - 1) // P
                agg_ps = psA.tile([P, win], F32, tag="aggT", name=f"agg{layer}_{w}")

                # ONE psum accumulation group per window (the start flag arms
                # zero-on-first-write for the whole 2KB bank; per-sub groups
                # would corrupt each other's partials). Map batch -> sub and
                # find the window's first/last batch for start/stop.
                sub_of_b = {}
                for blk in blocks:
                    if blk["w"] != w or blk["nb"] == 0:
                        continue
                    col = blk["col0"]
                    for t, nbt in blk["subs"]:
                        for bi in range(nbt):
                            sub_of_b[col + bi] = t
                        col += nbt
                win_first_b = min(sub_of_b)
                win_last_b = max(sub_of_b)

                # gather in <=GMAX-batch chunks (ucode: 1024 idxs/dma_gather),
                # consume each chunk's matmuls immediately (chunk-major order
                # keeps msg-pool pressure at 1-2 live tiles)
                for blk in blocks:
                    if blk["w"] != w or blk["nb"] == 0:
                        continue
                    tab = tab_lo if blk["ph"] == 0 else tab_hi
                    assert tab is not None, "hi-phase edges but no hi table"
                    for c0 in range(0, blk["nb"], GMAX):
                        cn = min(GMAX, blk["nb"] - c0)
                        msg = msgp.tile([P, GMAX, D], BF16, tag="msg",
                                        name=f"msg{layer}_{w}_{blk['ph']}_{c0}")
                        nidx = cn * P
                        b0 = blk["col0"] + c0
                        nc.gpsimd.dma_gather(
                            out_ap=msg[:, :cn, :],
                            in_ap=tab,
                            idxs_ap=idx_sb[:, b0 * 8:(b0 + cn) * 8],
                            num_idxs=nidx,
                            num_idxs_reg=nidx,
                            elem_size=D,
                        )
                        for bi in range(cn):
                            b = b0 + bi
                            t = sub_of_b[b]
                            j = t - w * spw
                            nsl = min(P, npc - t * P)
                            nc.tensor.matmul(
                                out=agg_ps[:, j * P:j * P + nsl],
                                lhsT=msg[:, bi, :],
                                rhs=segs[b // nb_onehot][:, b % nb_onehot, :nsl],
                                start=(b == win_first_b), stop=(b == win_last_b),
                            )

                aggTs = aggp.tile([P, win], F32, tag="aggTs",
                                  name=f"aggTs{layer}_{w}")
                nc.vector.tensor_tensor(
                    out=aggTs[:, :wn], in0=agg_ps[:, :wn],
                    in1=invcnt_sb[:, n0:n0 + wn], op=mybir.AluOpType.mult)

                if layer == 0:
                    ab_ps = psB.tile([P, win], F32, tag="AB", name=f"ab{w}")
                    nc.tensor.matmul(out=ab_ps[:, :wn], lhsT=W1l_sb[:],
                                     rhs=aggTs[:, :wn], start=True, stop=False)
                    nc.tensor.matmul(out=ab_ps[:, :wn], lhsT=W1r_sb[:],
                                     rhs=xT_sb[:, n0:n0 + wn], start=False, stop=True)
                    nc.scalar.activation(
                        out=hT_sb[:, n0:n0 + wn], in_=ab_ps[:, :wn],
                        func=mybir.ActivationFunctionType.Relu,
                        bias=b1_sb[:, 0:1], scale=1.0)
                    for j in range(nsw):
                        r0 = n0 + j * P
                        ns = min(P, npc - r0)
                        tr_ps = psT.tile([P, P], F32, tag="tr", name=f"tr{w}_{j}")
                        nc.tensor.transpose(out=tr_ps[:ns, :],
                                            in_=hT_sb[:, r0:r0 + ns],
                                            identity=ident_sb[:])
                        hrow = rowp.tile([P, D], BF16, tag="hrow",
                                         name=f"hrow{w}_{j}")
                        nc.vector.tensor_copy(out=hrow[:ns, :], in_=tr_ps[:ns, :])
                        nc.sync.dma_start(out=h_local[r0:r0 + ns, :],
                                          in_=hrow[:ns, :])
                else:
                    for j in range(nsw):
                        r0 = n0 + j * P
                        ns = min(P, npc - r0)
                        o_ps = psT.tile([P, P], F32, tag="tr", name=f"ops{w}_{j}")
                        nc.tensor.matmul(out=o_ps[:ns, :],
                                         lhsT=aggTs[:, j * P:j * P + ns],
                                         rhs=W2l_sb[:], start=True, stop=False)
                        nc.tensor.matmul(out=o_ps[:ns, :],
                                         lhsT=hT_sb[:, r0:r0 + ns],
                                         rhs=W2r_sb[:], start=False, stop=True)
                        orow = rowp.tile([P, D], F32, tag="orow",
                                         name=f"orow{w}_{j}")
                        nc.vector.tensor_tensor(
                            out=orow[:ns, :], in0=o_ps[:ns, :],
                            in1=b2row_sb[:ns, :], op=mybir.AluOpType.add)
                        nc.sync.dma_start(out=out_d[r0:r0 + ns, :],
                                          in_=orow[:ns, :])

        emit_layer(0, xtab)
        nc.gpsimd.collective_compute(
            "AllGather", mybir.AluOpType.bypass,
            replica_groups=[list(range(ncores))],
            ins=[h_local[:]], outs=[h_full[:]])
        emit_layer(1, h_full)

    nc.finalize()
    return nc


# ---------------------------------------------------------------- in_maps
def make_in_maps(x, edge_index, W1_l, b1_l, W1_r, W2_l, b2_l, W2_r,
                 n_nodes, ncores, win=512):
    prep = host_prep(edge_index, n_nodes, ncores, win=win)
    npc = prep["npc"]
    x = np.asarray(x, dtype=np.float32)
    xtab = x.astype(ml_dtypes.bfloat16)
    xT = np.ascontiguousarray(x.T)
    iota = np.tile(np.arange(P, dtype=np.float32)[None, :], (P, 1)).astype(
        ml_dtypes.bfloat16)
    ident = np.eye(P, dtype=np.float32)
    common = dict(
        xtab=xtab,
        W1l=np.asarray(W1_l, np.float32), W1r=np.asarray(W1_r, np.float32),
        W2l=np.asarray(W2_l, np.float32), W2r=np.asarray(W2_r, np.float32),
        b1=np.asarray(b1_l, np.float32).reshape(D, 1),
        b2row=np.tile(np.asarray(b2_l, np.float32).reshape(1, D), (P, 1)),
        iota=iota, ident=ident,
    )
    in_maps = []
    for c in range(ncores):
        in_maps.append(dict(
            common,
            xT=np.ascontiguousarray(xT[:, c * npc:(c + 1) * npc]),
            idx16=prep["idx16"][c], slots=prep["slots"][c],
            invcnt=np.tile(prep["invcnt"][c], (P, 1)),
        ))
    return prep, in_maps


# ------------------------------------------------------------------ kernel()
N_NODES = 50000
NCORES = 8

_cache = {}
last_result = None  # BassKernelResults of the most recent run (for test.py)


def kernel(x, edge_index, W1_l, b1_l, W1_r, W2_l, b2_l, W2_r,
           trace=False, trace_kwargs=None):
    """Full inputs in, full output out. Shards across 8 NeuronCores."""
    global last_result
    from concourse.bass_utils import run_bass_kernel_spmd

    x = np.asarray(x)
    edge_index = np.asarray(edge_index)
    n_nodes = x.shape[0]
    assert n_nodes % NCORES == 0

    prep, in_maps = make_in_maps(x, edge_index, W1_l, b1_l, W1_r,
                                 W2_l, b2_l, W2_r, n_nodes, NCORES)
    key = (n_nodes, prep["ncols"],
           tuple(blk["nb"] for blk in prep["blocks"]))
    if key not in _cache:
        _cache[key] = build_kernel(n_nodes, NCORES, prep)
    nc = _cache[key]

    res = run_bass_kernel_spmd(nc, in_maps, list(range(NCORES)),
                               trace=trace, **(trace_kwargs or {}))
    last_result = res
    out = np.concatenate([res.results[c]["out"] for c in range(NCORES)],
                         axis=0)
    return out.astype(np.float32)



# revision 4
# speedup vs baseline: 2.4248x; 2.4248x over previous
"""Trainium2 Bass kernel: 2-layer GraphSAGE (mean aggregation), 8-core SPMD.

nn_BiGNN: out = sage2(relu(sage1(x)));  sage(x) = mean_{j->i}(x_j) @ W_l + b_l + x @ W_r
N=50000 nodes, E=800000 edges, d=128, f32 inputs / f32 output.

Strategy (one NeuronCore owns 6250 destination nodes):
  - host: partition edges by destination block, sort by dst, pad per
    128-dst subwindow, split into src<32768 / src>=32768 phases (int16
    SWDGE index limit), equalize batch counts across cores (SPMD).
  - device: SWDGE dma_gather of bf16 source rows; one-hot segment matrices
    on DVE; TensorE matmul msg^T @ seg accumulated per 512-node PSUM
    window = transposed mean-aggregation; 1/deg scaling folded into the
    PSUM evacuation; weight matmuls + bias + relu; AllGather of h between
    layers; final layer emits row-major output directly.
"""

import os
import sys
import types

for _p in ("/opt/trn_rl_repo", "/root/.axon_site/_ro/trn_rl_repo",
           "/root/.axon_site"):
    if os.path.isdir(_p) and _p not in sys.path:
        sys.path.insert(0, _p)


def _install_ntff_hook():
    """Provide antenv.axon_hooks (missing in this image) so trace=True can
    capture NTFF profiles through libaxon_pjrt.so."""
    if "antenv.axon_hooks" in sys.modules:
        return
    store = [None]
    mod = types.ModuleType("antenv.axon_hooks")
    mod.set_axon_ntff_profile_hook = lambda h: store.__setitem__(0, h)
    mod.get_axon_ntff_profile_hook = lambda: store[0]
    sys.modules["antenv.axon_hooks"] = mod
    try:
        import antenv
        antenv.axon_hooks = mod
        from trn_agent_boot.trn_boot import _ntff_profile_via_ctypes
        so = "/opt/axon/libaxon_pjrt.so"
        if os.path.exists(so):
            mod.set_axon_ntff_profile_hook(_ntff_profile_via_ctypes(so))
    except Exception:
        pass


_install_ntff_hook()


import numpy as np
import ml_dtypes

import concourse.bass as bass
import concourse.bacc as bacc
import concourse.mybir as mybir
import concourse.tile as tile
from concourse.library_config import mlp as mlp_library

P = 128
D = 128
GMAX = 8  # max batches (1024 idxs) per dma_gather instruction
HALF = 32768  # int16 index limit for dma_gather
F32 = mybir.dt.float32
BF16 = mybir.dt.bfloat16
I16 = mybir.dt.int16


def wrap_idx16(arr):
    """[n] int array -> [128, n//16] int16 SWDGE layout (16-partition wrap,
    replicated for the 8 Q7 cores)."""
    n = arr.shape[0]
    assert n % 16 == 0
    w = np.asarray(arr, dtype=np.int16).reshape(n // 16, 16).T  # [16, n/16]
    return np.tile(w, (8, 1))  # [128, n/16]


# ----------------------------------------------------------------- host prep
def host_prep(edge_index, n_nodes, ncores, win=512):
    """Partition/sort/pad edges; build the phased stream + SWDGE idx arrays.

    Stream order: for each window w: [lo-phase batches][hi-phase batches],
    each phase grouped by subwindow. Per-(sub, phase) batch counts are
    equalized across cores so all cores share one program.
    """
    npc = n_nodes // ncores
    nsub = (npc + P - 1) // P
    nwin = (npc + win - 1) // win
    spw = win // P  # subwindows per window
    src_a = np.asarray(edge_index[0], dtype=np.int64)
    dst_a = np.asarray(edge_index[1], dtype=np.int64)

    # per (core, sub, phase) edge lists
    edges = [[None] * (2 * nsub) for _ in range(ncores)]
    invcnt = np.zeros((ncores, 1, npc), dtype=np.float32)
    for c in range(ncores):
        lo_n = c * npc
        m = (dst_a >= lo_n) & (dst_a < lo_n + npc)
        s, d = src_a[m], dst_a[m] - lo_n
        invcnt[c, 0] = 1.0 / np.maximum(np.bincount(d, minlength=npc), 1.0)
        order = np.argsort(d, kind="stable")
        s, d = s[order], d[order]
        sub = d // P
        for t in range(nsub):
            ms = sub == t
            st, dt_ = s[ms], d[ms]
            mlo = st < HALF
            edges[c][2 * t] = (st[mlo], dt_[mlo])               # lo phase
            edges[c][2 * t + 1] = (st[~mlo] - HALF, dt_[~mlo])  # hi phase

    # equalized batch counts per (sub, phase)
    nb = np.zeros((nsub, 2), dtype=np.int64)
    for c in range(ncores):
        for t in range(nsub):
            for ph in range(2):
                n = len(edges[c][2 * t + ph][0])
                nb[t, ph] = max(nb[t, ph], (n + P - 1) // P)
    nb[:, 0] = np.maximum(nb[:, 0], 1)  # ensure each sub has >=1 batch

    # stream layout: per window w, phase ph: one gather block
    blocks = []
    ncols = 0
    for w in range(nwin):
        subs = range(w * spw, min((w + 1) * spw, nsub))
        for ph in range(2):
            bl = [(t, int(nb[t, ph])) for t in subs]
            nbl = sum(x[1] for x in bl)
            blocks.append(dict(w=w, ph=ph, col0=ncols, nb=nbl, subs=bl))
            ncols += nbl

    # per-core data arrays in stream order
    idx16 = np.zeros((ncores, P, ncols * 8), dtype=np.int16)  # nb*128/16 = nb*8
    slots = np.full((ncores, P, ncols), -1.0, dtype=ml_dtypes.bfloat16)
    for c in range(ncores):
        for blk in blocks:
            if blk["nb"] == 0:
                continue
            col = blk["col0"]
            flat_idx = []
            for t, nbt in blk["subs"]:
                s, d = edges[c][2 * t + blk["ph"]]
                n = len(s)
                npad = nbt * P
                si = np.zeros(npad, dtype=np.int64)
                si[:n] = s
                sl = np.full(npad, -1.0, dtype=np.float32)
                sl[:n] = (d % P).astype(np.float32)
                for b in range(nbt):
                    slots[c, :, col + b] = sl[b * P:(b + 1) * P].astype(
                        ml_dtypes.bfloat16)
                flat_idx.append(si)
                col += nbt
            fi = np.concatenate(flat_idx)
            idx16[c, :, blk["col0"] * 8:(blk["col0"] + blk["nb"]) * 8] = \
                wrap_idx16(fi)

    return dict(npc=npc, nsub=nsub, nwin=nwin, win=win, ncols=ncols,
                blocks=blocks, idx16=idx16, slots=slots, invcnt=invcnt)


# -------------------------------------------------------------- kernel build
def build_kernel(n_nodes, ncores, prep, nb_onehot=8, fuse_onehot=False):
    npc, nwin, win = prep["npc"], prep["nwin"], prep["win"]
    ncols, blocks = prep["ncols"], prep["blocks"]
    spw = win // P

    nc = bacc.Bacc(None, num_swdge_queues=4)

    xtab = nc.declare_dram_parameter("xtab", [n_nodes, D], BF16, isOutput=False)
    xT_d = nc.declare_dram_parameter("xT", [D, npc], F32, isOutput=False)
    idx_d = nc.declare_dram_parameter("idx16", [P, ncols * 8], I16, isOutput=False)
    slots_d = nc.declare_dram_parameter("slots", [P, ncols], BF16, isOutput=False)
    invcnt_d = nc.declare_dram_parameter("invcnt", [P, npc], F32, isOutput=False)
    W1l_d = nc.declare_dram_parameter("W1l", [D, D], F32, isOutput=False)
    W1r_d = nc.declare_dram_parameter("W1r", [D, D], F32, isOutput=False)
    W2l_d = nc.declare_dram_parameter("W2l", [D, D], F32, isOutput=False)
    W2r_d = nc.declare_dram_parameter("W2r", [D, D], F32, isOutput=False)
    b1_d = nc.declare_dram_parameter("b1", [D, 1], F32, isOutput=False)
    b2row_d = nc.declare_dram_parameter("b2row", [P, D], F32, isOutput=False)
    iota_d = nc.declare_dram_parameter("iota", [P, P], BF16, isOutput=False)
    ident_d = nc.declare_dram_parameter("ident", [P, P], F32, isOutput=False)
    out_d = nc.declare_dram_parameter("out", [npc, D], F32, isOutput=True)


    from contextlib import ExitStack
    with tile.TileContext(nc) as tc, ExitStack() as es:
        dram = es.enter_context(tc.tile_pool(name="dram", bufs=1, space="DRAM"))
        h_local = dram.tile([npc, D], BF16, tag="hloc")
        h_full = dram.tile([n_nodes, D], BF16, tag="hfull", addr_space="Shared")

        const = es.enter_context(tc.tile_pool(name="const", bufs=1))
        sb = es.enter_context(tc.tile_pool(name="sb", bufs=1))
        msgp = es.enter_context(tc.tile_pool(name="msgp", bufs=6))
        segp = es.enter_context(tc.tile_pool(name="segp", bufs=4))
        aggp = es.enter_context(tc.tile_pool(name="aggp", bufs=2))
        rowp = es.enter_context(tc.tile_pool(name="rowp", bufs=3))
        psA = es.enter_context(tc.tile_pool(name="psA", bufs=2, space="PSUM"))
        psB = es.enter_context(tc.tile_pool(name="psB", bufs=2, space="PSUM"))
        psT = es.enter_context(tc.tile_pool(name="psT", bufs=2, space="PSUM"))

        nc.gpsimd.load_library(mlp_library)

        idx_sb = const.tile([P, ncols * 8], I16, tag="idx")
        slots_sb = const.tile([P, ncols], BF16, tag="slots")
        invcnt_sb = const.tile([P, npc], F32, tag="invcnt")
        iota_sb = const.tile([P, P], BF16, tag="iota")
        ident_sb = const.tile([P, P], F32, tag="ident")
        W1l_sb = const.tile([D, D], F32, tag="W1l")
        W1r_sb = const.tile([D, D], F32, tag="W1r")
        W2l_sb = const.tile([D, D], F32, tag="W2l")
        W2r_sb = const.tile([D, D], F32, tag="W2r")
        b1_sb = const.tile([D, 1], F32, tag="b1")
        b2row_sb = const.tile([P, D], F32, tag="b2row")
        xT_sb = sb.tile([D, npc], F32, tag="xT")
        hT_sb = sb.tile([D, npc], F32, tag="hT")

        for t, dd in [(idx_sb, idx_d), (slots_sb, slots_d), (invcnt_sb, invcnt_d),
                      (iota_sb, iota_d), (ident_sb, ident_d),
                      (W1l_sb, W1l_d), (W1r_sb, W1r_d), (W2l_sb, W2l_d),
                      (W2r_sb, W2r_d), (b1_sb, b1_d), (b2row_sb, b2row_d),
                      (xT_sb, xT_d)]:
            nc.sync.dma_start(out=t[:], in_=dd[:])

        ngrp = (ncols + nb_onehot - 1) // nb_onehot

        def emit_layer(layer, table):
            emit_layer.gq = 0
            segs = []
            for g in range(ngrp):
                nbg = min(nb_onehot, ncols - g * nb_onehot)
                seg = segp.tile([P, nb_onehot, P], BF16, tag="seg",
                                name=f"seg{layer}_{g}")
                g0 = g * nb_onehot
                if fuse_onehot:
                    nc.vector.tensor_tensor(
                        out=seg[:, :nbg, :],
                        in0=iota_sb[:, None, :].to_broadcast([P, nbg, P]),
                        in1=slots_sb[:, g0:g0 + nbg, None].to_broadcast([P, nbg, P]),
                        op=mybir.AluOpType.is_equal,
                    )
                else:
                    for bi in range(nbg):
                        nc.vector.tensor_tensor(
                            out=seg[:, bi, :],
                            in0=iota_sb[:],
                            in1=slots_sb[:, g0 + bi:g0 + bi + 1].to_broadcast([P, P]),
                            op=mybir.AluOpType.is_equal,
                        )
                segs.append(seg)

            tab_lo = table[0:min(HALF, n_nodes), :]
            tab_hi = table[HALF:n_nodes, :] if n_nodes > HALF else None

            for w in range(nwin):
                n0 = w * win
                wn = min(win, npc - n0)
                nsw = (wn + P - 1) // P
                agg_ps = psA.tile([P, win], F32, tag="aggT", name=f"agg{layer}_{w}")

                # ONE psum accumulation group per window (the start flag arms
                # zero-on-first-write for the whole 2KB bank; per-sub groups
                # would corrupt each other's partials). Map batch -> sub and
                # find the window's first/last batch for start/stop.
                sub_of_b = {}
                for blk in blocks:
                    if blk["w"] != w or blk["nb"] == 0:
                        continue
                    col = blk["col0"]
                    for t, nbt in blk["subs"]:
                        for bi in range(nbt):
                            sub_of_b[col + bi] = t
                        col += nbt
                win_first_b = min(sub_of_b)
                win_last_b = max(sub_of_b)

                # gather in <=GMAX-batch chunks (ucode: 1024 idxs/dma_gather),
                # consume each chunk's matmuls immediately (chunk-major order
                # keeps msg-pool pressure at 1-2 live tiles)
                for blk in blocks:
                    if blk["w"] != w or blk["nb"] == 0:
                        continue
                    tab = tab_lo if blk["ph"] == 0 else tab_hi
                    assert tab is not None, "hi-phase edges but no hi table"
                    for c0 in range(0, blk["nb"], GMAX):
                        cn = min(GMAX, blk["nb"] - c0)
                        msg = msgp.tile([P, GMAX, D], BF16, tag="msg",
                                        name=f"msg{layer}_{w}_{blk['ph']}_{c0}")
                        nidx = cn * P
                        b0 = blk["col0"] + c0
                        qn = emit_layer.gq
                        emit_layer.gq = (qn + 1) % 4
                        nc.gpsimd.dma_gather(
                            out_ap=msg[:, :cn, :],
                            in_ap=tab,
                            idxs_ap=idx_sb[:, b0 * 8:(b0 + cn) * 8],
                            num_idxs=nidx,
                            num_idxs_reg=nidx,
                            elem_size=D,
                            queue_num=qn,
                        )
                        for bi in range(cn):
                            b = b0 + bi
                            t = sub_of_b[b]
                            j = t - w * spw
                            nsl = min(P, npc - t * P)
                            nc.tensor.matmul(
                                out=agg_ps[:, j * P:j * P + nsl],
                                lhsT=msg[:, bi, :],
                                rhs=segs[b // nb_onehot][:, b % nb_onehot, :nsl],
                                start=(b == win_first_b), stop=(b == win_last_b),
                            )

                aggTs = aggp.tile([P, win], F32, tag="aggTs",
                                  name=f"aggTs{layer}_{w}")
                nc.vector.tensor_tensor(
                    out=aggTs[:, :wn], in0=agg_ps[:, :wn],
                    in1=invcnt_sb[:, n0:n0 + wn], op=mybir.AluOpType.mult)

                if layer == 0:
                    ab_ps = psB.tile([P, win], F32, tag="AB", name=f"ab{w}")
                    nc.tensor.matmul(out=ab_ps[:, :wn], lhsT=W1l_sb[:],
                                     rhs=aggTs[:, :wn], start=True, stop=False)
                    nc.tensor.matmul(out=ab_ps[:, :wn], lhsT=W1r_sb[:],
                                     rhs=xT_sb[:, n0:n0 + wn], start=False, stop=True)
                    nc.scalar.activation(
                        out=hT_sb[:, n0:n0 + wn], in_=ab_ps[:, :wn],
                        func=mybir.ActivationFunctionType.Relu,
                        bias=b1_sb[:, 0:1], scale=1.0)
                    for j in range(nsw):
                        r0 = n0 + j * P
                        ns = min(P, npc - r0)
                        tr_ps = psT.tile([P, P], F32, tag="tr", name=f"tr{w}_{j}")
                        nc.tensor.transpose(out=tr_ps[:ns, :],
                                            in_=hT_sb[:, r0:r0 + ns],
                                            identity=ident_sb[:])
                        hrow = rowp.tile([P, D], BF16, tag="hrow",
                                         name=f"hrow{w}_{j}")
                        nc.vector.tensor_copy(out=hrow[:ns, :], in_=tr_ps[:ns, :])
                        nc.sync.dma_start(out=h_local[r0:r0 + ns, :],
                                          in_=hrow[:ns, :])
                else:
                    for j in range(nsw):
                        r0 = n0 + j * P
                        ns = min(P, npc - r0)
                        o_ps = psT.tile([P, P], F32, tag="tr", name=f"ops{w}_{j}")
                        nc.tensor.matmul(out=o_ps[:ns, :],
                                         lhsT=aggTs[:, j * P:j * P + ns],
                                         rhs=W2l_sb[:], start=True, stop=False)
                        nc.tensor.matmul(out=o_ps[:ns, :],
                                         lhsT=hT_sb[:, r0:r0 + ns],
                                         rhs=W2r_sb[:], start=False, stop=True)
                        orow = rowp.tile([P, D], F32, tag="orow",
                                         name=f"orow{w}_{j}")
                        nc.vector.tensor_tensor(
                            out=orow[:ns, :], in0=o_ps[:ns, :],
                            in1=b2row_sb[:ns, :], op=mybir.AluOpType.add)
                        nc.sync.dma_start(out=out_d[r0:r0 + ns, :],
                                          in_=orow[:ns, :])

        emit_layer(0, xtab)
        nc.gpsimd.collective_compute(
            "AllGather", mybir.AluOpType.bypass,
            replica_groups=[list(range(ncores))],
            ins=[h_local[:]], outs=[h_full[:]])
        emit_layer(1, h_full)

    nc.finalize()
    return nc


# ---------------------------------------------------------------- in_maps
def make_in_maps(x, edge_index, W1_l, b1_l, W1_r, W2_l, b2_l, W2_r,
                 n_nodes, ncores, win=512):
    prep = host_prep(edge_index, n_nodes, ncores, win=win)
    npc = prep["npc"]
    x = np.asarray(x, dtype=np.float32)
    xtab = x.astype(ml_dtypes.bfloat16)
    xT = np.ascontiguousarray(x.T)
    iota = np.tile(np.arange(P, dtype=np.float32)[None, :], (P, 1)).astype(
        ml_dtypes.bfloat16)
    ident = np.eye(P, dtype=np.float32)
    common = dict(
        xtab=xtab,
        W1l=np.asarray(W1_l, np.float32), W1r=np.asarray(W1_r, np.float32),
        W2l=np.asarray(W2_l, np.float32), W2r=np.asarray(W2_r, np.float32),
        b1=np.asarray(b1_l, np.float32).reshape(D, 1),
        b2row=np.tile(np.asarray(b2_l, np.float32).reshape(1, D), (P, 1)),
        iota=iota, ident=ident,
    )
    in_maps = []
    for c in range(ncores):
        in_maps.append(dict(
            common,
            xT=np.ascontiguousarray(xT[:, c * npc:(c + 1) * npc]),
            idx16=prep["idx16"][c], slots=prep["slots"][c],
            invcnt=np.tile(prep["invcnt"][c], (P, 1)),
        ))
    return prep, in_maps


# ------------------------------------------------------------------ kernel()
N_NODES = 50000
NCORES = 8

_cache = {}
last_result = None  # BassKernelResults of the most recent run (for test.py)


def kernel(x, edge_index, W1_l, b1_l, W1_r, W2_l, b2_l, W2_r,
           trace=False, trace_kwargs=None):
    """Full inputs in, full output out. Shards across 8 NeuronCores."""
    global last_result
    from concourse.bass_utils import run_bass_kernel_spmd

    x = np.asarray(x)
    edge_index = np.asarray(edge_index)
    n_nodes = x.shape[0]
    assert n_nodes % NCORES == 0

    prep, in_maps = make_in_maps(x, edge_index, W1_l, b1_l, W1_r,
                                 W2_l, b2_l, W2_r, n_nodes, NCORES)
    key = (n_nodes, prep["ncols"],
           tuple(blk["nb"] for blk in prep["blocks"]))
    if key not in _cache:
        _cache[key] = build_kernel(n_nodes, NCORES, prep)
    nc = _cache[key]

    res = run_bass_kernel_spmd(nc, in_maps, list(range(NCORES)),
                               trace=trace, **(trace_kwargs or {}))
    last_result = res
    out = np.concatenate([res.results[c]["out"] for c in range(NCORES)],
                         axis=0)
    return out.astype(np.float32)

